# revision 50
# baseline (speedup 1.0000x reference)
"""Dot-product attention (no softmax) on 8 TRN2 NeuronCores.

out[b,h] = (q[b,h] @ k[b,h].T) @ v[b,h]  for q,k,v [B,H,L,D] = [2,16,2048,64] f32.

Strategy: matmul associativity -> out = q @ (k.T @ v). KV = k.T@v is [64,64]
per head, so the problem collapses from O(L^2 D) to O(L D^2) flops and becomes
purely memory bound (48 MiB in / 16 MiB out).

Sharding: the 32 (b,h) attention instances are independent; each of the 8
cores handles 4 consecutive heads of the flattened (b*h) axis. No collectives.

Per-core layout trick: a head's [2048, 64] tensor is viewed as [128, 16, 64]
(partition p holds rows 16p..16p+15, 4 KiB contiguous DRAM per partition, so
every DMA is fully coalesced). The KV reduction over L is order-independent,
and the same interleaved row mapping flows through transpose -> matmul ->
store unchanged.
"""

import sys

if "/opt/trn_rl_repo" not in sys.path:
    sys.path.insert(0, "/opt/trn_rl_repo")

from contextlib import ExitStack

import numpy as np

import os

import concourse.bass as bass
import concourse.tile as tile
from concourse import bacc, bass_utils, mybir
from concourse.bass_utils import run_bass_kernel_spmd
from concourse.masks import make_identity

if os.environ.get("ATTN_LDW_OPT") == "1" and not hasattr(bass_utils, "_attn_ldw_patch"):
    bass_utils._attn_ldw_patch = bass_utils.run_command

    def _run_command_ldw(cmd, *a, **kw):
        if isinstance(cmd, list):
            cmd = [
                "--enable-ldw-opt=true" if c == "--enable-ldw-opt=false" else c
                for c in cmd
            ]
        return bass_utils._attn_ldw_patch(cmd, *a, **kw)

    bass_utils.run_command = _run_command_ldw

B, H, L, D = 2, 16, 2048, 64
N_CORES = 8
HPC = (B * H) // N_CORES  # heads per core = 4
P = 128
J = L // P  # 16 row-slots per partition
F32 = mybir.dt.float32


def _body(ctx: ExitStack, tc: tile.TileContext, o_d, q_d, k_d, v_d):
    nc = tc.nc

    tag = "_ldw" if os.environ.get("ATTN_LDW_OPT") == "1" else ""
    const_pool = ctx.enter_context(tc.tile_pool(name="const" + tag, bufs=1))
    in_pool = ctx.enter_context(tc.tile_pool(name="in", bufs=4))
    qt_pool = ctx.enter_context(tc.tile_pool(name="qt", bufs=32))
    kv_pool = ctx.enter_context(tc.tile_pool(name="kv", bufs=4))
    out_pool = ctx.enter_context(tc.tile_pool(name="out", bufs=4))
    psum_kv = ctx.enter_context(tc.tile_pool(name="psum_kv", bufs=1, space="PSUM"))
    psum_s = ctx.enter_context(tc.tile_pool(name="psum_s", bufs=1, space="PSUM"))
    psum_t = ctx.enter_context(tc.tile_pool(name="psum_t", bufs=3, space="PSUM"))
    psum_o = ctx.enter_context(tc.tile_pool(name="psum_o", bufs=3, space="PSUM"))

    # HAM warm-up: dense bf16 matmuls while the first DMAs are in flight, so
    # the PE clock un-throttles (4/8 -> 8/8) before the real work. The memset
    # is the FIRST gpsimd op (ahead of q0's SWDGE descriptor generation) so
    # the burst starts ~1.2us in; q0's gating chunk goes out right after and
    # still lands before the first transpose needs it.
    warm_in = const_pool.tile([P, 4 * P], mybir.dt.bfloat16)
    nc.gpsimd.memset(warm_in[:], 0.0)

    q0_sb = in_pool.tile([P, J, D], F32, tag="q", name="q0")
    nc.gpsimd.dma_start(
        q0_sb[:, 0:2], q_d[0].rearrange("(p j) d -> p j d", p=P)[:, 0:2]
    )
    warm_ps = psum_o.tile([P, 4 * P], F32, tag="o_ps", name="warm_ps")
    for _ in range(8):
        nc.tensor.matmul(
            warm_ps[:], warm_in[:, 0:P], warm_in[:], start=True, stop=True
        )

    ident = const_pool.tile([P, P], F32)
    make_identity(nc, ident[:])

    # ones_dbl[p, m] = 1 iff p == m (mod 64): one matmul against it both sums
    # the two column-tiled KV halves and replicates the result to partitions
    # 64..127 (needed as the row-group-1 operand of the row-tiled out matmuls).
    ones_dbl = const_pool.tile([P, P], F32)
    nc.gpsimd.memset(ones_dbl[:], 0.0)
    for off in (-64, 0, 64):
        nc.gpsimd.affine_select(
            out=ones_dbl[:],
            in_=ones_dbl[:],
            compare_op=mybir.AluOpType.not_equal,
            fill=1.0,
            base=-off,
            pattern=[[-1, P]],
            channel_multiplier=1,
        )

    # Interleaved load schedule: q of heads 0..1 first (transposes depend only
    # on q, so the PE has work from the first 256 KiB on), then per head k/v
    # with q prefetched two heads ahead.
    q_sbs, k_sbs, v_sbs = [], [], []
    for h in range(HPC):
        q_sbs.append(
            q0_sb if h == 0 else in_pool.tile([P, J, D], F32, tag="q", name=f"q{h}")
        )
        k_sbs.append(in_pool.tile([P, J, D], F32, tag="k", name=f"k{h}"))
        v_sbs.append(in_pool.tile([P, J, D], F32, tag="v", name=f"v{h}"))

    def load_q(h, stagger=False):
        qv = q_d[h].rearrange("(p j) d -> p j d", p=P)
        if stagger:  # slots 0:2 already in flight from the kernel prologue
            nc.sync.dma_start(q_sbs[h][:, 2 : J // 2], qv[:, 2 : J // 2])
            nc.sync.dma_start(q_sbs[h][:, J // 2 : J], qv[:, J // 2 : J])
        else:
            nc.sync.dma_start(q_sbs[h][:], qv[:])

    def load_k(h):
        nc.sync.dma_start(k_sbs[h][:], k_d[h].rearrange("(p j) d -> p j d", p=P))

    def load_v(h):
        nc.sync.dma_start(v_sbs[h][:], v_d[h].rearrange("(p j) d -> p j d", p=P))

    # Interleave so KV_0's operands land just as the transposes run out.
    load_q(0, stagger=True)
    load_k(0)
    if HPC > 1:
        load_q(1)
    load_v(0)
    for h in range(2, HPC):
        load_q(h)
        load_k(h - 1)
        load_v(h - 1)
    if HPC > 1:
        load_k(HPC - 1)
        load_v(HPC - 1)

    # Software-pipelined emission: every head's transpose + KV + KV2 chain is
    # emitted before any O phase, so the cross-engine kv2 chains (PSUM copy ->
    # ones_dbl matmul -> kv2 copies) hide under other heads' PE work instead
    # of stalling it — in particular the last head's chain is not exposed at
    # the kernel tail.
    qts_all, kv2s = [], []
    for h in range(HPC):
        q_sb, k_sb, v_sb = q_sbs[h], k_sbs[h], v_sbs[h]

        qts = []
        for jp in range(J // 2):
            qt_ps = psum_t.tile([P, P], F32, tag="qt_ps")
            nc.tensor.transpose(qt_ps[:], q_sb[:, 2 * jp : 2 * jp + 2], ident[:])
            qt_sb = qt_pool.tile([P, P], F32, tag="qt", name=f"qt{h}_{jp}")
            nc.scalar.activation(
                qt_sb[:], qt_ps[:], mybir.ActivationFunctionType.Identity
            )
            qts.append(qt_sb)
        qts_all.append(qts)

        # KV = k.T @ v, column-tiled: even j-slots accumulate into PE columns
        # 0..63 (psum partitions 0..63), odd slots into columns 64..127, so
        # the two matmuls of a pair run concurrently in the array.
        kv_ps = psum_kv.tile([P, D], F32)
        for jp in range(J // 2):
            nc.tensor.matmul(
                kv_ps[0:D],
                k_sb[:, 2 * jp],
                v_sb[:, 2 * jp],
                start=(jp == 0),
                stop=(jp == J // 2 - 1),
                tile_position=(0, 0),
                skip_group_check=True,
            )
            nc.tensor.matmul(
                kv_ps[D : 2 * D],
                k_sb[:, 2 * jp + 1],
                v_sb[:, 2 * jp + 1],
                start=(jp == 0),
                stop=(jp == J // 2 - 1),
                tile_position=(0, D),
                skip_group_check=True,
            )
        kv_raw = kv_pool.tile([P, D], F32, tag="kv_raw", name=f"kvr{h}")
        nc.vector.tensor_copy(kv_raw[:], kv_ps[:])
        kv_st_ps = psum_s.tile([P, D], F32, tag="kv_st", name=f"kvs{h}")
        nc.tensor.matmul(kv_st_ps[:], ones_dbl[:], kv_raw[:], start=True, stop=True)
        # KV2 = blockdiag(KV, KV): one [128,128] matmul against it computes two
        # output slots at once (lhsT = a transposed q slab pair).
        kv2 = kv_pool.tile([P, P], F32, tag="kv2", name=f"kv2_{h}")
        nc.gpsimd.memset(kv2[:], 0.0)
        nc.vector.tensor_copy(kv2[0:D, 0:D], kv_st_ps[0:D])
        nc.vector.tensor_copy(kv2[D : 2 * D, D : 2 * D], kv_st_ps[D : 2 * D])
        kv2s.append(kv2)

    for h in range(HPC):
        out_sb = out_pool.tile([P, J, D], F32, tag="o", name=f"o{h}")
        ov = o_d[h].rearrange("(p j) d -> p j d", p=P)
        kv2 = kv2s[h]
        for jp in range(J // 2):
            o_ps = psum_o.tile([P, 2, D], F32, tag="o_ps")
            nc.tensor.matmul(o_ps[:], qts_all[h][jp][:], kv2[:], start=True, stop=True)
            nc.vector.tensor_copy(out_sb[:, 2 * jp : 2 * jp + 2], o_ps[:])
            if h == HPC - 1:
                # last head: store per pair-of-slots so the ~2us HBM
                # completion receipts of the final DMAs overlap
                if jp % 2 == 1:
                    sl = slice(2 * jp - 2, 2 * jp + 2)
                    nc.sync.dma_start(ov[:, sl], out_sb[:, sl])
            elif jp == J // 4 - 1:
                nc.sync.dma_start(ov[:, 0 : J // 2], out_sb[:, 0 : J // 2])
        if h != HPC - 1:
            nc.sync.dma_start(ov[:, J // 2 : J], out_sb[:, J // 2 : J])


def build():
    nc = bacc.Bacc("TRN2", target_bir_lowering=False, debug=False)
    q_d = nc.dram_tensor("q", [HPC, L, D], F32, kind="ExternalInput").ap()
    k_d = nc.dram_tensor("k", [HPC, L, D], F32, kind="ExternalInput").ap()
    v_d = nc.dram_tensor("v", [HPC, L, D], F32, kind="ExternalInput").ap()
    o_d = nc.dram_tensor("out", [HPC, L, D], F32, kind="ExternalOutput").ap()
    with tile.TileContext(nc) as tc, ExitStack() as ctx:
        _body(ctx, tc, o_d, q_d, k_d, v_d)
    nc.compile()
    return nc


_NC = None


def _get_nc():
    global _NC
    if _NC is None:
        _NC = build()
    return _NC


def make_in_maps(q, k, v):
    qf = np.ascontiguousarray(np.asarray(q, dtype=np.float32).reshape(B * H, L, D))
    kf = np.ascontiguousarray(np.asarray(k, dtype=np.float32).reshape(B * H, L, D))
    vf = np.ascontiguousarray(np.asarray(v, dtype=np.float32).reshape(B * H, L, D))
    return [
        {
            "q": np.ascontiguousarray(qf[c * HPC : (c + 1) * HPC]),
            "k": np.ascontiguousarray(kf[c * HPC : (c + 1) * HPC]),
            "v": np.ascontiguousarray(vf[c * HPC : (c + 1) * HPC]),
        }
        for c in range(N_CORES)
    ]


def run_sharded(q, k, v, **spmd_kwargs):
    """Run on all 8 cores; returns (full_output, BassKernelResults)."""
    nc = _get_nc()
    res = run_bass_kernel_spmd(
        nc, make_in_maps(q, k, v), core_ids=list(range(N_CORES)), **spmd_kwargs
    )
    shards = [np.asarray(res.results[c]["out"]) for c in range(N_CORES)]
    out = np.concatenate(shards, axis=0).reshape(B, H, L, D).astype(np.float32)
    return out, res


def kernel(q, k, v):
    out, _ = run_sharded(q, k, v)
    return out


# revision 51
# speedup vs baseline: 1.0058x; 1.0058x over previous
"""Dot-product attention (no softmax) on 8 TRN2 NeuronCores.

out[b,h] = (q[b,h] @ k[b,h].T) @ v[b,h]  for q,k,v [B,H,L,D] = [2,16,2048,64] f32.

Strategy: matmul associativity -> out = q @ (k.T @ v). KV = k.T@v is [64,64]
per head, so the problem collapses from O(L^2 D) to O(L D^2) flops and becomes
purely memory bound (48 MiB in / 16 MiB out).

Sharding: the 32 (b,h) attention instances are independent; each of the 8
cores handles 4 consecutive heads of the flattened (b*h) axis. No collectives.

Per-core layout trick: a head's [2048, 64] tensor is viewed as [128, 16, 64]
(partition p holds rows 16p..16p+15, 4 KiB contiguous DRAM per partition, so
every DMA is fully coalesced). The KV reduction over L is order-independent,
and the same interleaved row mapping flows through transpose -> matmul ->
store unchanged.
"""

import sys

if "/opt/trn_rl_repo" not in sys.path:
    sys.path.insert(0, "/opt/trn_rl_repo")

from contextlib import ExitStack

import numpy as np

import os

import concourse.bass as bass
import concourse.tile as tile
from concourse import bacc, bass_utils, mybir
from concourse.bass_utils import run_bass_kernel_spmd
from concourse.masks import make_identity

if os.environ.get("ATTN_LDW_OPT") == "1" and not hasattr(bass_utils, "_attn_ldw_patch"):
    bass_utils._attn_ldw_patch = bass_utils.run_command

    def _run_command_ldw(cmd, *a, **kw):
        if isinstance(cmd, list):
            cmd = [
                "--enable-ldw-opt=true" if c == "--enable-ldw-opt=false" else c
                for c in cmd
            ]
        return bass_utils._attn_ldw_patch(cmd, *a, **kw)

    bass_utils.run_command = _run_command_ldw

B, H, L, D = 2, 16, 2048, 64
N_CORES = 8
HPC = (B * H) // N_CORES  # heads per core = 4
P = 128
J = L // P  # 16 row-slots per partition
F32 = mybir.dt.float32


def _body(ctx: ExitStack, tc: tile.TileContext, o_d, q_d, k_d, v_d):
    nc = tc.nc

    tag = "_ldw" if os.environ.get("ATTN_LDW_OPT") == "1" else ""
    const_pool = ctx.enter_context(tc.tile_pool(name="const" + tag, bufs=1))
    in_pool = ctx.enter_context(tc.tile_pool(name="in", bufs=4))
    qt_pool = ctx.enter_context(tc.tile_pool(name="qt", bufs=32))
    kv_pool = ctx.enter_context(tc.tile_pool(name="kv", bufs=4))
    out_pool = ctx.enter_context(tc.tile_pool(name="out", bufs=4))
    psum_kv = ctx.enter_context(tc.tile_pool(name="psum_kv", bufs=1, space="PSUM"))
    psum_s = ctx.enter_context(tc.tile_pool(name="psum_s", bufs=1, space="PSUM"))
    psum_t = ctx.enter_context(tc.tile_pool(name="psum_t", bufs=3, space="PSUM"))
    psum_o = ctx.enter_context(tc.tile_pool(name="psum_o", bufs=3, space="PSUM"))

    # q0's first 64 KiB goes out immediately: first op on the gpsimd queue
    # (no register-load preamble, ahead of the const memsets), so the first
    # transpose can start ~1.5us in.
    q0_sb = in_pool.tile([P, J, D], F32, tag="q", name="q0")
    nc.gpsimd.dma_start(
        q0_sb[:, 0:2], q_d[0].rearrange("(p j) d -> p j d", p=P)[:, 0:2]
    )

    # HAM warm-up: ~4us of dense bf16 matmuls while the first DMAs are in
    # flight, so the PE clock un-throttles (4/8 -> 8/8) before the real
    # transposes/matmuls start. Results are never read.
    warm_in = const_pool.tile([P, 4 * P], mybir.dt.bfloat16)
    nc.gpsimd.memset(warm_in[:], 0.0)
    warm_ps = psum_o.tile([P, 4 * P], F32, tag="o_ps", name="warm_ps")
    for _ in range(8):
        nc.tensor.matmul(
            warm_ps[:], warm_in[:, 0:P], warm_in[:], start=True, stop=True
        )

    ident = const_pool.tile([P, P], F32)
    make_identity(nc, ident[:])

    # ones_dbl[p, m] = 1 iff p == m (mod 64): one matmul against it both sums
    # the two column-tiled KV halves and replicates the result to partitions
    # 64..127 (needed as the row-group-1 operand of the row-tiled out matmuls).
    ones_dbl = const_pool.tile([P, P], F32)
    nc.gpsimd.memset(ones_dbl[:], 0.0)
    for off in (-64, 0, 64):
        nc.gpsimd.affine_select(
            out=ones_dbl[:],
            in_=ones_dbl[:],
            compare_op=mybir.AluOpType.not_equal,
            fill=1.0,
            base=-off,
            pattern=[[-1, P]],
            channel_multiplier=1,
        )

    # Interleaved load schedule: q of heads 0..1 first (transposes depend only
    # on q, so the PE has work from the first 256 KiB on), then per head k/v
    # with q prefetched two heads ahead.
    q_sbs, k_sbs, v_sbs = [], [], []
    for h in range(HPC):
        q_sbs.append(
            q0_sb if h == 0 else in_pool.tile([P, J, D], F32, tag="q", name=f"q{h}")
        )
        k_sbs.append(in_pool.tile([P, J, D], F32, tag="k", name=f"k{h}"))
        v_sbs.append(in_pool.tile([P, J, D], F32, tag="v", name=f"v{h}"))

    def load_q(h, stagger=False):
        qv = q_d[h].rearrange("(p j) d -> p j d", p=P)
        if stagger:  # slots 0:2 already in flight from the kernel prologue
            nc.sync.dma_start(q_sbs[h][:, 2 : J // 2], qv[:, 2 : J // 2])
            nc.sync.dma_start(q_sbs[h][:, J // 2 : J], qv[:, J // 2 : J])
        else:
            nc.sync.dma_start(q_sbs[h][:], qv[:])

    def load_k(h):
        nc.sync.dma_start(k_sbs[h][:], k_d[h].rearrange("(p j) d -> p j d", p=P))

    def load_v(h):
        nc.sync.dma_start(v_sbs[h][:], v_d[h].rearrange("(p j) d -> p j d", p=P))

    # Interleave so KV_0's operands land just as the transposes run out.
    load_q(0, stagger=True)
    load_k(0)
    if HPC > 1:
        load_q(1)
    load_v(0)
    for h in range(2, HPC):
        load_q(h)
        load_k(h - 1)
        load_v(h - 1)
    if HPC > 1:
        load_k(HPC - 1)
        load_v(HPC - 1)

    # Software-pipelined emission: every head's transpose + KV + KV2 chain is
    # emitted before any O phase, so the cross-engine kv2 chains (PSUM copy ->
    # ones_dbl matmul -> kv2 copies) hide under other heads' PE work instead
    # of stalling it — in particular the last head's chain is not exposed at
    # the kernel tail.
    qts_all, kv2s = [], []
    for h in range(HPC):
        q_sb, k_sb, v_sb = q_sbs[h], k_sbs[h], v_sbs[h]

        qts = []
        for jp in range(J // 2):
            qt_ps = psum_t.tile([P, P], F32, tag="qt_ps")
            nc.tensor.transpose(qt_ps[:], q_sb[:, 2 * jp : 2 * jp + 2], ident[:])
            qt_sb = qt_pool.tile([P, P], F32, tag="qt", name=f"qt{h}_{jp}")
            nc.scalar.activation(
                qt_sb[:], qt_ps[:], mybir.ActivationFunctionType.Identity
            )
            qts.append(qt_sb)
        qts_all.append(qts)

        # KV = k.T @ v, column-tiled: even j-slots accumulate into PE columns
        # 0..63 (psum partitions 0..63), odd slots into columns 64..127, so
        # the two matmuls of a pair run concurrently in the array.
        kv_ps = psum_kv.tile([P, D], F32)
        for jp in range(J // 2):
            nc.tensor.matmul(
                kv_ps[0:D],
                k_sb[:, 2 * jp],
                v_sb[:, 2 * jp],
                start=(jp == 0),
                stop=(jp == J // 2 - 1),
                tile_position=(0, 0),
                skip_group_check=True,
            )
            nc.tensor.matmul(
                kv_ps[D : 2 * D],
                k_sb[:, 2 * jp + 1],
                v_sb[:, 2 * jp + 1],
                start=(jp == 0),
                stop=(jp == J // 2 - 1),
                tile_position=(0, D),
                skip_group_check=True,
            )
        kv_raw = kv_pool.tile([P, D], F32, tag="kv_raw", name=f"kvr{h}")
        nc.vector.tensor_copy(kv_raw[:], kv_ps[:])
        kv_st_ps = psum_s.tile([P, D], F32, tag="kv_st", name=f"kvs{h}")
        nc.tensor.matmul(kv_st_ps[:], ones_dbl[:], kv_raw[:], start=True, stop=True)
        # KV2 = blockdiag(KV, KV): one [128,128] matmul against it computes two
        # output slots at once (lhsT = a transposed q slab pair).
        kv2 = kv_pool.tile([P, P], F32, tag="kv2", name=f"kv2_{h}")
        nc.gpsimd.memset(kv2[:], 0.0)
        nc.vector.tensor_copy(kv2[0:D, 0:D], kv_st_ps[0:D])
        nc.vector.tensor_copy(kv2[D : 2 * D, D : 2 * D], kv_st_ps[D : 2 * D])
        kv2s.append(kv2)

    for h in range(HPC):
        out_sb = out_pool.tile([P, J, D], F32, tag="o", name=f"o{h}")
        ov = o_d[h].rearrange("(p j) d -> p j d", p=P)
        kv2 = kv2s[h]
        for jp in range(J // 2):
            o_ps = psum_o.tile([P, 2, D], F32, tag="o_ps")
            nc.tensor.matmul(o_ps[:], qts_all[h][jp][:], kv2[:], start=True, stop=True)
            nc.vector.tensor_copy(out_sb[:, 2 * jp : 2 * jp + 2], o_ps[:])
            if h == HPC - 1:
                # last head: store per pair-of-slots so the ~2us HBM
                # completion receipts of the final DMAs overlap
                if jp % 2 == 1:
                    sl = slice(2 * jp - 2, 2 * jp + 2)
                    nc.sync.dma_start(ov[:, sl], out_sb[:, sl])
            elif jp == J // 4 - 1:
                nc.sync.dma_start(ov[:, 0 : J // 2], out_sb[:, 0 : J // 2])
        if h != HPC - 1:
            nc.sync.dma_start(ov[:, J // 2 : J], out_sb[:, J // 2 : J])


def build():
    nc = bacc.Bacc("TRN2", target_bir_lowering=False, debug=False)
    q_d = nc.dram_tensor("q", [HPC, L, D], F32, kind="ExternalInput").ap()
    k_d = nc.dram_tensor("k", [HPC, L, D], F32, kind="ExternalInput").ap()
    v_d = nc.dram_tensor("v", [HPC, L, D], F32, kind="ExternalInput").ap()
    o_d = nc.dram_tensor("out", [HPC, L, D], F32, kind="ExternalOutput").ap()
    with tile.TileContext(nc) as tc, ExitStack() as ctx:
        _body(ctx, tc, o_d, q_d, k_d, v_d)
    nc.compile()
    return nc


_NC = None


def _get_nc():
    global _NC
    if _NC is None:
        _NC = build()
    return _NC


def make_in_maps(q, k, v):
    qf = np.ascontiguousarray(np.asarray(q, dtype=np.float32).reshape(B * H, L, D))
    kf = np.ascontiguousarray(np.asarray(k, dtype=np.float32).reshape(B * H, L, D))
    vf = np.ascontiguousarray(np.asarray(v, dtype=np.float32).reshape(B * H, L, D))
    return [
        {
            "q": np.ascontiguousarray(qf[c * HPC : (c + 1) * HPC]),
            "k": np.ascontiguousarray(kf[c * HPC : (c + 1) * HPC]),
            "v": np.ascontiguousarray(vf[c * HPC : (c + 1) * HPC]),
        }
        for c in range(N_CORES)
    ]


def run_sharded(q, k, v, **spmd_kwargs):
    """Run on all 8 cores; returns (full_output, BassKernelResults)."""
    nc = _get_nc()
    res = run_bass_kernel_spmd(
        nc, make_in_maps(q, k, v), core_ids=list(range(N_CORES)), **spmd_kwargs
    )
    shards = [np.asarray(res.results[c]["out"]) for c in range(N_CORES)]
    out = np.concatenate(shards, axis=0).reshape(B, H, L, D).astype(np.float32)
    return out, res


def kernel(q, k, v):
    out, _ = run_sharded(q, k, v)
    return out
